# revision 3
# baseline (speedup 1.0000x reference)
"""LoRA layer kernel for 8 Trainium2 NeuronCores.

Computes out = x @ W^T + bias + SCALE * (x @ B) @ A
  x: (4, 2048, 4096) f32, W: (4096, 4096), bias: (4096,), B: (4096, 16), A: (16, 4096)

Strategy: data-parallel over tokens (8192 tokens -> 1024/core). Each core:
  - keeps its x^T shard [4096(d), 1024(m)] resident in SBUF (read from HBM once)
  - streams W^T [4096(d), 4096(n)] column tiles (read once)
  - computes transposed output tiles out^T[n(128 part), m(512 free)] in PSUM via
    float32r matmuls (full PE rate at free-dim>=256), LoRA second matmul (K=16)
    accumulates into the same PSUM group, bias added on ScalarE during the
    PSUM->SBUF drain (bias varies along partitions in this layout).
Host side: transpose x/W once with numpy, scale A by SCALE, gather + untranspose.
"""

import numpy as np
from contextlib import ExitStack

import concourse.tile as tile
from concourse import bacc, mybir
from concourse.bass import ds, ts
from concourse.bass_utils import run_bass_kernel_spmd

N_CORES = 8
D = 4096            # in_features (contraction)
K = 4096            # out_features
M_TOTAL = 8192      # tokens
M = M_TOTAL // N_CORES
R = 16              # LoRA rank
SCALE = 32.0 / 16.0
P = 128
DCH = D // P        # 32 contraction chunks
N_TILES = K // P    # 32 output-feature tiles
M_CH = 512          # moving free-dim per matmul
N_MCH = M // M_CH   # 2

F32 = mybir.dt.float32
F32R = mybir.dt.float32r

LAST_RESULTS = None  # set on every call; test harness reads exec_time_ns from it


def _build():
    nc = bacc.Bacc("TRN2", target_bir_lowering=False, debug=False,
                   num_devices=N_CORES)
    xT_d = nc.dram_tensor("xT", [D, M], F32R, kind="ExternalInput").ap()
    wT_d = nc.dram_tensor("wT", [D, K], F32R, kind="ExternalInput").ap()
    biasT_d = nc.dram_tensor("biasT", [P, N_TILES], F32, kind="ExternalInput").ap()
    b_d = nc.dram_tensor("B", [D, R], F32R, kind="ExternalInput").ap()
    a_d = nc.dram_tensor("A", [R, K], F32R, kind="ExternalInput").ap()
    outT_d = nc.dram_tensor("outT", [K, M], F32, kind="ExternalOutput").ap()

    xT_v = xT_d.rearrange("(c p) m -> c p m", p=P)      # [DCH, P, M]
    wT_v = wT_d.rearrange("(c p) k -> c p k", p=P)      # [DCH, P, K]
    b_v = b_d.rearrange("(c p) r -> c p r", p=P)        # [DCH, P, R]
    outT_v = outT_d.rearrange("(t p) m -> t p m", p=P)  # [N_TILES, P, M]

    with tile.TileContext(nc) as tc, ExitStack() as ctx:
        const = ctx.enter_context(tc.tile_pool(name="const", bufs=1))
        wpool = ctx.enter_context(tc.tile_pool(name="w", bufs=2))
        opool = ctx.enter_context(tc.tile_pool(name="o", bufs=3))
        ppool = ctx.enter_context(tc.tile_pool(name="ps", bufs=4, space="PSUM"))
        tpool = ctx.enter_context(tc.tile_pool(name="tps", bufs=2, space="PSUM"))

        # resident inputs
        x_sb = const.tile([P, DCH, M], F32R)
        for d in range(DCH):
            nc.sync.dma_start(out=x_sb[:, d, :], in_=xT_v[d])
        b_sb = const.tile([P, DCH, R], F32R)
        for d in range(DCH):
            nc.sync.dma_start(out=b_sb[:, d, :], in_=b_v[d])
        a_sb = const.tile([R, K], F32R)
        nc.sync.dma_start(out=a_sb[:], in_=a_d[:])
        bias_sb = const.tile([P, N_TILES], F32)
        nc.sync.dma_start(out=bias_sb[:], in_=biasT_d[:])

        # stage 1: t^T = (x @ B)^T  -> [R, M] in SBUF (SCALE folded into A on host)
        tT_sb = const.tile([R, M], F32R)
        for mc in range(N_MCH):
            pt = tpool.tile([R, M_CH], F32)
            for d in range(DCH):
                nc.tensor.matmul(pt[:], lhsT=b_sb[:, d, :],
                                 rhs=x_sb[:, d, ts(mc, M_CH)],
                                 start=(d == 0), stop=(d == DCH - 1))
            nc.vector.tensor_copy(out=tT_sb[:, ts(mc, M_CH)], in_=pt[:])

        # stage 2: base matmul + LoRA second matmul + bias
        for n in range(N_TILES):
            w_t = wpool.tile([P, DCH, P], F32R)
            for d in range(DCH):
                nc.sync.dma_start(out=w_t[:, d, :], in_=wT_v[d, :, ds(n * P, P)])
            for mc in range(N_MCH):
                ps = ppool.tile([P, M_CH], F32)
                for d in range(DCH):
                    nc.tensor.matmul(ps[:], lhsT=w_t[:, d, :],
                                     rhs=x_sb[:, d, ts(mc, M_CH)],
                                     start=(d == 0), stop=False)
                nc.tensor.matmul(ps[:], lhsT=a_sb[:, ds(n * P, P)],
                                 rhs=tT_sb[:, ts(mc, M_CH)],
                                 start=False, stop=True)
                ot = opool.tile([P, M_CH], F32)
                nc.scalar.add(ot[:], ps[:], bias_sb[:, ds(n, 1)])
                nc.sync.dma_start(out=outT_v[n][:, ts(mc, M_CH)], in_=ot[:])

    nc.compile()
    return nc


_NC = None


def kernel(x, W, bias, B, A):
    global _NC, LAST_RESULTS
    if _NC is None:
        _NC = _build()

    xT = np.ascontiguousarray(x.reshape(M_TOTAL, D).T)          # [D, M_TOTAL]
    wT = np.ascontiguousarray(W.T)                              # [D, K]
    biasT = np.ascontiguousarray(bias.reshape(N_TILES, P).T)    # [P, N_TILES]
    a_s = np.ascontiguousarray(SCALE * A.astype(np.float32))    # [R, K]
    b_c = np.ascontiguousarray(B.astype(np.float32))

    in_maps = []
    for c in range(N_CORES):
        in_maps.append({
            "xT": np.ascontiguousarray(xT[:, c * M:(c + 1) * M]),
            "wT": wT,
            "biasT": biasT,
            "B": b_c,
            "A": a_s,
        })

    res = run_bass_kernel_spmd(_NC, in_maps, list(range(N_CORES)))
    LAST_RESULTS = res
    outT = np.concatenate([res.results[c]["outT"] for c in range(N_CORES)],
                          axis=1)                               # [K, M_TOTAL]
    out = np.ascontiguousarray(outT.T).reshape(4, 2048, 4096)
    return out.astype(np.float32)


# revision 8
# speedup vs baseline: 1.4446x; 1.4446x over previous
"""LoRA layer kernel for 8 Trainium2 NeuronCores.

Computes out = x @ W^T + bias + SCALE * (x @ B) @ A
  x: (4, 2048, 4096) f32, W: (4096, 4096), bias: (4096,), B: (4096, 16), A: (16, 4096)

Strategy: data-parallel over tokens (8192 tokens -> 1024/core). Each core:
  - keeps its x^T shard [4096(d), 1024(m)] resident in SBUF (read from HBM once)
  - streams W^T [4096(d), 4096(n)] column tiles (read once)
  - computes transposed output tiles out^T[n(128 part), m(512 free)] in PSUM via
    float32r matmuls (full PE rate at free-dim>=256), LoRA second matmul (K=16)
    accumulates into the same PSUM group, bias added on ScalarE during the
    PSUM->SBUF drain (bias varies along partitions in this layout).
Host side: transpose x/W once with numpy, scale A by SCALE, gather + untranspose.
"""

import numpy as np
from contextlib import ExitStack

import concourse.tile as tile
from concourse import bacc, mybir
from concourse.bass import ds, ts
from concourse.bass_utils import run_bass_kernel_spmd

N_CORES = 8
D = 4096            # in_features (contraction)
K = 4096            # out_features
M_TOTAL = 8192      # tokens
M = M_TOTAL // N_CORES
R = 16              # LoRA rank
SCALE = 32.0 / 16.0
P = 128
DCH = D // P        # 32 contraction chunks
N_TILES = K // P    # 32 output-feature tiles
M_CH = 512          # moving free-dim per matmul
N_MCH = M // M_CH   # 2

F32 = mybir.dt.float32
F32R = mybir.dt.float32r

LAST_RESULTS = None  # set on every call; test harness reads exec_time_ns from it


def _build():
    nc = bacc.Bacc("TRN2", target_bir_lowering=False, debug=False,
                   num_devices=N_CORES)
    xT_d = nc.dram_tensor("xT", [D, M], F32R, kind="ExternalInput").ap()
    wT_d = nc.dram_tensor("wT", [D, K], F32R, kind="ExternalInput").ap()
    biasT_d = nc.dram_tensor("biasT", [P, N_TILES], F32, kind="ExternalInput").ap()
    b_d = nc.dram_tensor("B", [D, R], F32R, kind="ExternalInput").ap()
    a_d = nc.dram_tensor("A", [R, K], F32R, kind="ExternalInput").ap()
    outT_d = nc.dram_tensor("outT", [K, M], F32, kind="ExternalOutput").ap()

    xT_v = xT_d.rearrange("(c p) m -> c p m", p=P)      # [DCH, P, M]
    wT_v = wT_d.rearrange("(c p) k -> p c k", p=P)      # [P, DCH, K] (p-major walk)
    b_v = b_d.rearrange("(c p) r -> p c r", p=P)        # [P, DCH, R]
    outT_v = outT_d.rearrange("(t p) m -> t p m", p=P)  # [N_TILES, P, M]

    with tile.TileContext(nc) as tc, ExitStack() as ctx:
        const = ctx.enter_context(tc.tile_pool(name="const", bufs=1))
        wpool = ctx.enter_context(tc.tile_pool(name="w", bufs=2))
        opool = ctx.enter_context(tc.tile_pool(name="o", bufs=3))
        ppool = ctx.enter_context(tc.tile_pool(name="ps", bufs=3, space="PSUM"))
        tpool = ctx.enter_context(tc.tile_pool(name="tps", bufs=2, space="PSUM"))

        # resident inputs; x as per-chunk tiles so matmuls depend on single
        # chunk DMAs (PE starts as soon as chunk 0 lands, not after 16.8MB)
        x_sb = []
        for d in range(DCH):
            xc = const.tile([P, M], F32R, tag=f"x{d}", name=f"x{d}")
            nc.gpsimd.dma_start(out=xc[:], in_=xT_v[d])
            x_sb.append(xc)
        b_sb = const.tile([P, DCH, R], F32R)
        for d4 in range(0, DCH, 8):
            nc.gpsimd.dma_start(out=b_sb[:, ds(d4, 8), :],
                                in_=b_v[:, ds(d4, 8), :])
        a_sb = const.tile([R, K], F32R)
        nc.gpsimd.dma_start(out=a_sb[:], in_=a_d[:])
        bias_sb = const.tile([P, N_TILES], F32)
        nc.gpsimd.dma_start(out=bias_sb[:], in_=biasT_d[:])

        # stage 1: t^T = (x @ B)^T  -> [R, M] in SBUF (SCALE folded into A on host)
        tT_sb = const.tile([R, M], F32R)
        for mc in range(N_MCH):
            pt = tpool.tile([R, M_CH], F32)
            for d in range(DCH):
                nc.tensor.matmul(pt[:], lhsT=b_sb[:, d, :],
                                 rhs=x_sb[d][:, ts(mc, M_CH)],
                                 start=(d == 0), stop=(d == DCH - 1))
            nc.vector.tensor_copy(out=tT_sb[:, ts(mc, M_CH)], in_=pt[:])

        # stage 2: base matmul + LoRA second matmul + bias
        for n in range(N_TILES):
            w_t = wpool.tile([P, DCH, P], F32R)
            for d4 in range(0, DCH, 4):
                nc.sync.dma_start(out=w_t[:, ds(d4, 4), :],
                                  in_=wT_v[:, ds(d4, 4), ds(n * P, P)])
            pss = [ppool.tile([P, M_CH], F32, tag=f"ps{mc}", name=f"ps{mc}")
                   for mc in range(N_MCH)]
            for d in range(DCH):
                for mc in range(N_MCH):
                    nc.tensor.matmul(pss[mc][:], lhsT=w_t[:, d, :],
                                     rhs=x_sb[d][:, ts(mc, M_CH)],
                                     start=(d == 0), stop=False)
            for mc in range(N_MCH):
                nc.tensor.matmul(pss[mc][:], lhsT=a_sb[:, ds(n * P, P)],
                                 rhs=tT_sb[:, ts(mc, M_CH)],
                                 start=False, stop=True)
                ot = opool.tile([P, M_CH], F32)
                nc.vector.tensor_scalar_add(ot[:], pss[mc][:], bias_sb[:, ds(n, 1)])
                nc.sync.dma_start(out=outT_v[n][:, ts(mc, M_CH)], in_=ot[:])

    nc.compile()
    return nc


_NC = None


def kernel(x, W, bias, B, A):
    global _NC, LAST_RESULTS
    if _NC is None:
        _NC = _build()

    xT = np.ascontiguousarray(x.reshape(M_TOTAL, D).T)          # [D, M_TOTAL]
    wT = np.ascontiguousarray(W.T)                              # [D, K]
    biasT = np.ascontiguousarray(bias.reshape(N_TILES, P).T)    # [P, N_TILES]
    a_s = np.ascontiguousarray(SCALE * A.astype(np.float32))    # [R, K]
    b_c = np.ascontiguousarray(B.astype(np.float32))

    in_maps = []
    for c in range(N_CORES):
        in_maps.append({
            "xT": np.ascontiguousarray(xT[:, c * M:(c + 1) * M]),
            "wT": wT,
            "biasT": biasT,
            "B": b_c,
            "A": a_s,
        })

    res = run_bass_kernel_spmd(_NC, in_maps, list(range(N_CORES)))
    LAST_RESULTS = res
    outT = np.concatenate([res.results[c]["outT"] for c in range(N_CORES)],
                          axis=1)                               # [K, M_TOTAL]
    out = np.ascontiguousarray(outT.T).reshape(4, 2048, 4096)
    return out.astype(np.float32)
